# revision 7
# baseline (speedup 1.0000x reference)
"""Trainium2 Bass kernel v3: sequence-data-parallel decoder, f32r matmuls.

Same sharding as v2 (8 cores = 2 batch x 4 ranks, core owns token chunks
(r, 7-r) of its batch; full weights streamed per core; one KV AllGather per
layer). v3 keys off two hardware findings:
  - bf16 matmuls cost ~5us each on this stack (ldweights slow path);
    float32r runs at full rate -> every matmul operand is f32r in SBUF.
  - weights stay bf16 in DRAM and are cast-DMA'd (gpsimd SWDGE) to f32
    SBUF, halving HBM traffic vs f32 weights.
  - a DMA that starts waiting on a collective semaphore before it is set
    pays a ~2.5ms poll penalty -> the KV unpack is queued behind the
    layer's weight-stream DMAs on the same (gpsimd) queue.
"""
import os
import sys

sys.path.insert(0, "/opt/trn_rl_repo")

import numpy as np
import ml_dtypes

L, B, T, HID = 4, 2, 1024, 2048
NH, NKV, HD = 16, 4, 128
INTER = 5632
EPS = 1e-6
NCORES, RANKS = 8, 4
KT = HID // 128
NQT = NH * HD // 128
NKVT = NKV * HD // 128
NCT = NQT + 2 * NKVT
NIT = INTER // 128
CH = 128
TOK = 2 * CH
NKC = 8
RG = [[0, 1, 2, 3], [4, 5, 6, 7]]
WCAST = os.environ.get("K3_WCAST", "1") == "1"  # bf16 weights + cast DMA

_CACHE = {}


def _chunks(r):
    return (r, 7 - r)


def _build_program(with_bias, depth_mult=1, coll_mode="ag"):
    import concourse.bacc as bacc
    import concourse.tile as tile
    import concourse.mybir as mybir
    from contextlib import ExitStack

    F32 = mybir.dt.float32
    F32R = mybir.dt.float32r
    BF16 = mybir.dt.bfloat16
    WDT = BF16 if WCAST else F32R
    AF = mybir.ActivationFunctionType
    OP = mybir.AluOpType

    nc = bacc.Bacc("TRN2", target_bir_lowering=False, debug=False,
                   num_devices=NCORES)

    def wdma(out, in_):
        # weight-stream DMA: casting (gpsimd SWDGE) when DRAM is bf16
        if WCAST:
            nc.gpsimd.dma_start(out=out, in_=in_)
        else:
            nc.gpsimd.dma_start(out=out, in_=in_)

    XT = nc.dram_tensor("xt_in", [HID, TOK], F32, kind="ExternalInput")
    WQKV = nc.dram_tensor("wqkv", [L, NCT, 128, KT, 128], WDT, kind="ExternalInput")
    WO = nc.dram_tensor("wo", [L, KT, 128, KT, 128], WDT, kind="ExternalInput")
    WGU = nc.dram_tensor("wgu", [L, NIT, 128, KT, 256], WDT, kind="ExternalInput")
    WD = nc.dram_tensor("wd", [L, KT, 128, NIT, 128], WDT, kind="ExternalInput")
    COST = nc.dram_tensor("cost", [HD, TOK], F32, kind="ExternalInput")
    SINST = nc.dram_tensor("sinst", [HD, TOK], F32, kind="ExternalInput")
    AMASK = nc.dram_tensor("amask", [NKC, 128, 2 * TOK], BF16, kind="ExternalInput")
    COS4 = nc.dram_tensor("cos4", [HD, 4 * TOK], F32, kind="ExternalInput")
    SIN4 = nc.dram_tensor("sin4", [HD, 4 * TOK], F32, kind="ExternalInput")
    ROT = nc.dram_tensor("rot", [128, 128], F32R, kind="ExternalInput")
    IDT = nc.dram_tensor("idt", [128, 128], F32R, kind="ExternalInput")
    ONES = nc.dram_tensor("ones", [128, 1], F32R, kind="ExternalInput")
    NRMW = nc.dram_tensor("nrmw", [128, KT], F32, kind="ExternalInput")
    EPST = nc.dram_tensor("epst", [1, 1], F32, kind="ExternalInput")
    if with_bias:
        QKVB = nc.dram_tensor("qkvb", [L, 128, NCT], F32, kind="ExternalInput")
    OXT = nc.dram_tensor("oxt", [HID, TOK], F32, kind="ExternalOutput")

    with tile.TileContext(nc) as tc, ExitStack() as top:
        persist = top.enter_context(tc.tile_pool(name="persist", bufs=1))
        dram = top.enter_context(tc.tile_pool(name="dram", bufs=2, space="DRAM"))

        xt = persist.tile([128, KT, TOK], F32)
        nc.sync.dma_start(out=xt, in_=XT.ap().rearrange("(k p) t -> p k t", p=128))
        cost = persist.tile([128, TOK], F32)
        nc.sync.dma_start(out=cost, in_=COST.ap())
        sinst = persist.tile([128, TOK], F32)
        nc.sync.dma_start(out=sinst, in_=SINST.ap())
        amask = persist.tile([128, NKC, 2 * TOK], BF16)
        nc.sync.dma_start(out=amask, in_=AMASK.ap().rearrange("g p t -> p g t"))
        cos4 = persist.tile([128, 4 * TOK], F32)
        nc.sync.dma_start(out=cos4, in_=COS4.ap())
        sin4 = persist.tile([128, 4 * TOK], F32)
        nc.sync.dma_start(out=sin4, in_=SIN4.ap())
        rotm = persist.tile([128, 128], F32R)
        nc.sync.dma_start(out=rotm, in_=ROT.ap())
        idt = persist.tile([128, 128], F32R)
        nc.sync.dma_start(out=idt, in_=IDT.ap())
        ones = persist.tile([128, 1], F32R)
        nc.sync.dma_start(out=ones, in_=ONES.ap())
        nrmw = persist.tile([128, KT], F32)
        nc.sync.dma_start(out=nrmw, in_=NRMW.ap())
        epst = persist.tile([1, 1], F32)
        nc.sync.dma_start(out=epst, in_=EPST.ap())
        if with_bias:
            qkvb = persist.tile([128, L, NCT], F32)
            nc.sync.dma_start(out=qkvb, in_=QKVB.ap().rearrange("l p c -> p l c"))

        def norm_to(pool, psp, xh):
            """xh [128, KT, TOK] f32r = x * rsqrt(mean(x^2)+eps)."""
            var = psp.tile([1, TOK], F32, name="var", bufs=1)
            for k in range(KT):
                sq = pool.tile([128, TOK], F32R, name="sq", bufs=3)
                nc.vector.tensor_tensor(out=sq, in0=xt[:, k, :], in1=xt[:, k, :],
                                        op=OP.mult)
                nc.tensor.matmul(var, ones, sq, start=(k == 0), stop=(k == KT - 1),
                                 skip_group_check=True)
            std = pool.tile([1, TOK], F32, name="std", bufs=1)
            nc.scalar.activation(out=std, in_=var, func=AF.Sqrt,
                                 bias=epst[:, 0:1], scale=1.0 / HID)
            rec = pool.tile([1, TOK], F32, name="rec", bufs=1)
            nc.vector.reciprocal(out=rec, in_=std)
            rb = pool.tile([128, TOK], F32, name="rb", bufs=1)
            nc.gpsimd.partition_broadcast(rb, rec)
            for k in range(KT):
                nc.vector.tensor_tensor(out=xh[:, k, :], in0=xt[:, k, :],
                                        in1=rb, op=OP.mult)

        for l in [li % L for li in range(L * depth_mult)]:
            with ExitStack() as ls:
                sbL = ls.enter_context(tc.tile_pool(name="sbL", bufs=1))
                aoT = sbL.tile([128, NQT // 2, 2, TOK], F32R, name="aoT")
                att_scope = ExitStack()
                sbQK = att_scope.enter_context(tc.tile_pool(name="sbQK", bufs=1))
                qf = sbQK.tile([128, NQT // 2, 2, TOK], F32R, name="qf")
                kall = sbQK.tile([128, NKV, RANKS, TOK], F32R, name="kall")
                vall = sbQK.tile([128, NKV, RANKS, TOK], F32R, name="vall")

                # ---------- phase A: norm1 + kv/q proj + rope + AG ----------
                with ExitStack() as ph:
                    sbA = ph.enter_context(tc.tile_pool(name="sbA", bufs=2))
                    psS = ph.enter_context(tc.tile_pool(name="psS", bufs=1, space="PSUM"))
                    psW = ph.enter_context(tc.tile_pool(name="psW", bufs=3, space="PSUM"))
                    psR = ph.enter_context(tc.tile_pool(name="psR", bufs=2, space="PSUM"))

                    xh = sbA.tile([128, KT, TOK], F32R, name="xh", bufs=1)
                    norm_to(sbA, psS, xh)

                    kvpack = sbA.tile([128, NKV, 2, TOK], F32R, name="kvpack", bufs=1)

                    def proj(ct, wtile):
                        ps = psW.tile([128, TOK], F32, name="pqkv", bufs=3)
                        for k in range(KT):
                            nc.tensor.matmul(ps, wtile[:, k, :], xh[:, k, :],
                                             start=(k == 0), stop=(k == KT - 1),
                                             skip_group_check=True)
                        return ps

                    def rope4(dst4, raw4):
                        # raw4 [128, 4, TOK] f32r staged; dst4 same-shape AP
                        rps = psR.tile([128, 2, 2 * TOK], F32, name="rps", bufs=1)
                        for hh in range(2):
                            nc.tensor.matmul(rps[:, hh], rotm,
                                             raw4[:, 2 * hh:2 * hh + 2, :],
                                             start=True, stop=True,
                                             skip_group_check=True)
                        qc = sbA.tile([128, 4 * TOK], F32, name="qc", bufs=2)
                        nc.vector.tensor_tensor(out=qc, in0=raw4.bitcast(F32),
                                                in1=cos4, op=OP.mult)
                        rs = sbA.tile([128, 4 * TOK], F32, name="rs", bufs=2)
                        nc.vector.tensor_tensor(out=rs, in0=rps, in1=sin4, op=OP.mult)
                        nc.vector.tensor_tensor(out=dst4, in0=qc, in1=rs, op=OP.add)

                    # k tiles: proj + staged rope -> kvpack[:, :, 0, :]
                    kraw = sbA.tile([128, NKVT, TOK], F32R, name="kraw", bufs=1)
                    for j in range(NKVT):
                        ct = NQT + j
                        w = sbA.tile([128, KT, 128], F32R, name="wqkv_t", bufs=3)
                        wdma(w, WQKV.ap()[l, ct])
                        ps = proj(ct, w)
                        if with_bias:
                            nc.vector.tensor_scalar_add(
                                out=kraw[:, j, :], in0=ps,
                                scalar1=qkvb[:, l, ct:ct + 1])
                        else:
                            nc.scalar.copy(kraw[:, j, :], ps)
                    rope4(kvpack[:, :, 0, :], kraw)
                    # v tiles: proj + evict + transpose -> kvpack[:, j, 1, :]
                    for j in range(NKVT):
                        ct = NQT + NKVT + j
                        w = sbA.tile([128, KT, 128], F32R, name="wqkv_t", bufs=3)
                        wdma(w, WQKV.ap()[l, ct])
                        ps = proj(ct, w)
                        vtmp = sbA.tile([128, TOK], F32R, name="vtmp", bufs=2)
                        if with_bias:
                            nc.vector.tensor_scalar_add(
                                out=vtmp, in0=ps, scalar1=qkvb[:, l, ct:ct + 1])
                        else:
                            nc.scalar.copy(vtmp, ps)
                        for cc in range(2):
                            pv = psR.tile([128, 128], F32R, name="pv", bufs=1)
                            nc.tensor.transpose(
                                pv, vtmp[:, cc * 128:(cc + 1) * 128], idt)
                            nc.scalar.copy(kvpack[:, j, 1, cc * 128:(cc + 1) * 128],
                                           pv.bitcast(F32))

                    # KV exchange: SBUF f32 -> DRAM bf16 (cast) -> AllGather
                    kvout = dram.tile([RANKS, 128, NKV, 2, TOK], BF16, name="kvout",
                                      bufs=2)
                    kvin = dram.tile([128, NKV, 2, TOK], BF16, name="kvin", bufs=2)
                    nc.gpsimd.dma_start(out=kvin, in_=kvpack.bitcast(F32))
                    if coll_mode == "fake":
                        for rr in range(RANKS):
                            nc.gpsimd.dma_start(out=kvout[rr], in_=kvin)
                    else:
                        nc.gpsimd.collective_compute(
                            "AllGather", mybir.AluOpType.bypass, replica_groups=RG,
                            ins=[kvin.opt()], outs=[kvout.opt()])

                    # q tiles + rope; their weight DMAs double as the queue
                    # spacer between AG launch and the unpack below
                    qraw = None
                    for ct in range(NQT):
                        w = sbA.tile([128, KT, 128], F32R, name="wqkv_t", bufs=3)
                        wdma(w, WQKV.ap()[l, ct])
                        ps = proj(ct, w)
                        if ct % 4 == 0:
                            qraw = sbA.tile([128, 4, TOK], F32R, name="qraw", bufs=2)
                        if with_bias:
                            nc.vector.tensor_scalar_add(
                                out=qraw[:, ct % 4, :], in0=ps,
                                scalar1=qkvb[:, l, ct:ct + 1])
                        else:
                            nc.scalar.copy(qraw[:, ct % 4, :], ps)
                        if ct % 4 == 3:
                            g4 = ct // 4
                            rope4(qf[:, 2 * g4:2 * g4 + 2, :, :], qraw)

                    # unpack gathered KV (cast bf16 -> f32), first AG waiter
                    for kv in range(NKV):
                        nc.gpsimd.dma_start(
                            out=kall[:, kv], in_=kvout[:, :, kv, 0, :].rearrange(
                                "r p t -> p r t"))
                        nc.gpsimd.dma_start(
                            out=vall[:, kv], in_=kvout[:, :, kv, 1, :].rearrange(
                                "r p t -> p r t"))

                # ---------- phase B: attention ----------
                with ExitStack() as ph:
                    sbB = ph.enter_context(tc.tile_pool(name="sbB", bufs=2))
                    psSc = ph.enter_context(tc.tile_pool(name="psSc", bufs=3, space="PSUM"))
                    psAO = ph.enter_context(tc.tile_pool(name="psAO", bufs=2, space="PSUM"))
                    psSum = ph.enter_context(tc.tile_pool(name="psSum", bufs=2, space="PSUM"))

                    for hp in range(NQT // 2):  # head pair
                        kv = hp // 2
                        pao = psAO.tile([128, 2 * TOK], F32, name="pao", bufs=2)
                        psm = psSum.tile([1, 2 * TOK], F32, name="psm", bufs=2)
                        for g in range(NKC):
                            rr, cc = (g, 0) if g < 4 else (7 - g, 1)
                            ktile = kall[:, kv, rr, cc * 128:(cc + 1) * 128]
                            vtile = vall[:, kv, rr, cc * 128:(cc + 1) * 128]
                            sc = psSc.tile([128, 2 * TOK], F32, name="sc", bufs=3)
                            nc.tensor.matmul(sc, ktile, qf[:, hp, :, :],
                                             start=True, stop=True,
                                             skip_group_check=True)
                            ex = sbB.tile([128, 2 * TOK], F32R, name="ex", bufs=4)
                            nc.scalar.activation(out=ex, in_=sc, func=AF.Exp)
                            nc.vector.tensor_tensor(out=ex, in0=ex.bitcast(F32),
                                                    in1=amask[:, g, :], op=OP.mult)
                            nc.tensor.matmul(pao, vtile, ex,
                                             start=(g == 0), stop=(g == NKC - 1),
                                             skip_group_check=True)
                            nc.tensor.matmul(psm, ones, ex,
                                             start=(g == 0), stop=(g == NKC - 1),
                                             skip_group_check=True)
                        rw = sbB.tile([1, 2 * TOK], F32, name="rw", bufs=2)
                        nc.vector.reciprocal(out=rw, in_=psm)
                        rb = sbB.tile([128, 2 * TOK], F32, name="rb2", bufs=2)
                        nc.gpsimd.partition_broadcast(rb, rw)
                        nc.vector.tensor_tensor(out=aoT[:, hp, :, :], in0=pao,
                                                in1=rb, op=OP.mult)
                att_scope.close()

                # ---------- phase C: o-proj + residual ----------
                with ExitStack() as ph:
                    sbC = ph.enter_context(tc.tile_pool(name="sbC", bufs=2))
                    psO = ph.enter_context(tc.tile_pool(name="psO", bufs=2, space="PSUM"))
                    for ho in range(KT):
                        w = sbC.tile([128, KT, 128], F32R, name="wo_t", bufs=3)
                        wdma(w, WO.ap()[l, ho])
                        po = psO.tile([128, TOK], F32, name="po", bufs=2)
                        for hk in range(NQT):
                            nc.tensor.matmul(po, w[:, hk, :],
                                             aoT[:, hk // 2, hk % 2, :],
                                             start=(hk == 0), stop=(hk == NQT - 1),
                                             skip_group_check=True)
                        nc.vector.tensor_tensor(out=xt[:, ho, :], in0=xt[:, ho, :],
                                                in1=po, op=OP.add)

                # ---------- phase D: norm2 + MLP + residual ----------
                with ExitStack() as ph:
                    sbD = ph.enter_context(tc.tile_pool(name="sbD", bufs=2))
                    psS2 = ph.enter_context(tc.tile_pool(name="psS2", bufs=1, space="PSUM"))
                    psG = ph.enter_context(tc.tile_pool(name="psG", bufs=2, space="PSUM"))
                    psU = ph.enter_context(tc.tile_pool(name="psU", bufs=2, space="PSUM"))
                    psD = ph.enter_context(tc.tile_pool(name="psD", bufs=2, space="PSUM"))

                    xh2 = sbD.tile([128, KT, TOK], F32R, name="xh2", bufs=1)
                    norm_to(sbD, psS2, xh2)
                    mT = sbD.tile([128, NIT, TOK], F32R, name="mT", bufs=1)
                    for ci in range(NIT):
                        wgu = sbD.tile([128, KT, 256], F32R, name="wgu", bufs=2)
                        wdma(wgu, WGU.ap()[l, ci])
                        pg = psG.tile([128, TOK], F32, name="pg", bufs=2)
                        pu = psU.tile([128, TOK], F32, name="pu", bufs=2)
                        for k in range(KT):
                            nc.tensor.matmul(pg, wgu[:, k, 0:128], xh2[:, k, :],
                                             start=(k == 0), stop=(k == KT - 1),
                                             skip_group_check=True)
                        for k in range(KT):
                            nc.tensor.matmul(pu, wgu[:, k, 128:256], xh2[:, k, :],
                                             start=(k == 0), stop=(k == KT - 1),
                                             skip_group_check=True)
                        gsl = sbD.tile([128, TOK], F32, name="gsl", bufs=2)
                        nc.scalar.activation(out=gsl, in_=pg, func=AF.Silu)
                        nc.vector.tensor_tensor(out=mT[:, ci, :], in0=gsl, in1=pu,
                                                op=OP.mult)
                    for ho in range(KT):
                        wd = sbD.tile([128, NIT, 128], F32R, name="wd_t", bufs=2)
                        wdma(wd, WD.ap()[l, ho])
                        pd = psD.tile([128, TOK], F32, name="pd", bufs=2)
                        for ki in range(NIT):
                            nc.tensor.matmul(pd, wd[:, ki, :], mT[:, ki, :],
                                             start=(ki == 0), stop=(ki == NIT - 1),
                                             skip_group_check=True)
                        nc.vector.tensor_tensor(out=xt[:, ho, :], in0=xt[:, ho, :],
                                                in1=pd, op=OP.add)

        # ---------------- final norm + output ----------------
        with ExitStack() as ph:
            sbF = ph.enter_context(tc.tile_pool(name="sbF", bufs=2))
            psF = ph.enter_context(tc.tile_pool(name="psF", bufs=1, space="PSUM"))
            var = psF.tile([1, TOK], F32, name="var", bufs=1)
            for k in range(KT):
                sq = sbF.tile([128, TOK], F32R, name="sq", bufs=3)
                nc.vector.tensor_tensor(out=sq, in0=xt[:, k, :], in1=xt[:, k, :],
                                        op=OP.mult)
                nc.tensor.matmul(var, ones, sq, start=(k == 0), stop=(k == KT - 1),
                                 skip_group_check=True)
            std = sbF.tile([1, TOK], F32, name="std", bufs=1)
            nc.scalar.activation(out=std, in_=var, func=AF.Sqrt,
                                 bias=epst[:, 0:1], scale=1.0 / HID)
            rec = sbF.tile([1, TOK], F32, name="rec", bufs=1)
            nc.vector.reciprocal(out=rec, in_=std)
            rb = sbF.tile([128, TOK], F32, name="rbf", bufs=1)
            nc.gpsimd.partition_broadcast(rb, rec)
            for k in range(KT):
                tmp = sbF.tile([128, TOK], F32, name="tmp", bufs=3)
                nc.vector.tensor_tensor(out=tmp, in0=xt[:, k, :], in1=rb, op=OP.mult)
                ot = sbF.tile([128, TOK], F32, name="ot", bufs=3)
                nc.vector.tensor_scalar_mul(out=ot, in0=tmp, scalar1=nrmw[:, k:k + 1])
                nc.sync.dma_start(out=OXT.ap()[k * 128:(k + 1) * 128, :], in_=ot)

    nc.compile()
    return nc


def _prepare_inputs(inputs):
    g = {k: np.asarray(v) for k, v in inputs.items()}
    qw, kw, vw, ow = g["qw"], g["kw"], g["vw"], g["ow"]
    gatew, upw, downw = g["gatew"], g["upw"], g["downw"]
    ln1w, ln2w, normw = g["ln1w"], g["ln2w"], g["normw"]
    hs, cos, sin = g["hidden_states"], g["cos"], g["sin"]
    qb, kb, vb = g["qb"], g["kb"], g["vb"]

    with_bias = bool(np.any(qb) or np.any(kb) or np.any(vb))
    sc = 1.0 / np.sqrt(HD)
    wdt = ml_dtypes.bfloat16 if WCAST else np.float32

    wqkv = np.empty([L, NCT, 128, KT, 128], wdt)
    wo = np.empty([L, KT, 128, KT, 128], wdt)
    wgu = np.empty([L, NIT, 128, KT, 256], wdt)
    wd = np.empty([L, KT, 128, NIT, 128], wdt)
    qkvb = np.zeros([L, 128, NCT], np.float32)

    def pmajor(wt):
        K = wt.shape[0]
        return np.ascontiguousarray(
            wt.reshape(K // 128, 128, wt.shape[1]).transpose(1, 0, 2))

    for l in range(L):
        qs = (qw[l] * ln1w[l][None, :] * sc).astype(np.float32)
        ks = (kw[l] * ln1w[l][None, :]).astype(np.float32)
        vs = (vw[l] * ln1w[l][None, :]).astype(np.float32)
        gs = (gatew[l] * ln2w[l][None, :]).astype(np.float32)
        us = (upw[l] * ln2w[l][None, :]).astype(np.float32)
        for j in range(NQT):
            wqkv[l, j] = pmajor(qs[j * 128:(j + 1) * 128, :].T).astype(wdt)
            qkvb[l, :, j] = qb[l, j * 128:(j + 1) * 128] * sc
        for j in range(NKVT):
            wqkv[l, NQT + j] = pmajor(ks[j * 128:(j + 1) * 128, :].T).astype(wdt)
            wqkv[l, NQT + NKVT + j] = pmajor(
                vs[j * 128:(j + 1) * 128, :].T).astype(wdt)
            qkvb[l, :, NQT + j] = kb[l, j * 128:(j + 1) * 128]
            qkvb[l, :, NQT + NKVT + j] = vb[l, j * 128:(j + 1) * 128]
        for j in range(KT):
            wo[l, j] = pmajor(ow[l].T[:, j * 128:(j + 1) * 128]).astype(wdt)
        gut = np.empty([HID, 256], np.float32)
        for ci in range(NIT):
            gut[:, 0:128] = gs[ci * 128:(ci + 1) * 128, :].T
            gut[:, 128:256] = us[ci * 128:(ci + 1) * 128, :].T
            wgu[l, ci] = pmajor(gut).astype(wdt)
        for j in range(KT):
            wd[l, j] = pmajor(downw[l].T[:, j * 128:(j + 1) * 128]).astype(wdt)

    rotm = np.zeros([128, 128], np.float32)
    for i in range(64):
        rotm[i, i + 64] = 1.0
        rotm[i + 64, i] = 1.0

    common = {
        "wqkv": wqkv, "wo": wo, "wgu": wgu, "wd": wd,
        "rot": rotm,
        "idt": np.eye(128, dtype=np.float32),
        "ones": np.ones([128, 1], np.float32),
        "nrmw": np.ascontiguousarray(
            np.asarray(normw, np.float32).reshape(KT, 128).T),
        "epst": np.full([1, 1], EPS, np.float32),
    }
    if with_bias:
        common["qkvb"] = qkvb

    in_maps = []
    for c in range(NCORES):
        b, r = c // RANKS, c % RANKS
        c0, c1 = _chunks(r)
        idx = np.concatenate([np.arange(c0 * CH, (c0 + 1) * CH),
                              np.arange(c1 * CH, (c1 + 1) * CH)])
        m = dict(common)
        m["xt_in"] = np.ascontiguousarray(np.asarray(hs[b], np.float32).T[:, idx])
        m["cost"] = np.ascontiguousarray(np.asarray(cos[b], np.float32).T[:, idx])
        sb_ = np.asarray(sin[b], np.float32)
        m["sinst"] = np.ascontiguousarray(
            np.concatenate([-sb_[idx, :HD // 2].T, sb_[idx, HD // 2:].T], axis=0))
        kpos = np.arange(CH)
        am = np.empty([NKC, 128, TOK], np.float32)
        for gch in range(NKC):
            kabs = gch * CH + kpos
            am[gch] = (kabs[:, None] <= idx[None, :]).astype(np.float32)
        m["amask"] = np.ascontiguousarray(
            np.concatenate([am, am], axis=2).astype(ml_dtypes.bfloat16))
        m["cos4"] = np.ascontiguousarray(np.tile(m["cost"], (1, 4)))
        m["sin4"] = np.ascontiguousarray(np.tile(m["sinst"], (1, 4)))
        in_maps.append(m)
    return in_maps, with_bias


def _get_program(with_bias, depth_mult=1, fake_coll=False, coll_mode=None):
    cm = "fake" if fake_coll else (coll_mode or "ag")
    key = ("prog3", with_bias, depth_mult, cm, WCAST)
    if key not in _CACHE:
        _CACHE[key] = _build_program(with_bias, depth_mult, cm)
    return _CACHE[key]


def assemble(results):
    out = np.empty([B, T, HID], np.float32)
    for c in range(NCORES):
        b, r = c // RANKS, c % RANKS
        c0, c1 = _chunks(r)
        idx = np.concatenate([np.arange(c0 * CH, (c0 + 1) * CH),
                              np.arange(c1 * CH, (c1 + 1) * CH)])
        out[b, idx, :] = np.asarray(results[c]["oxt"], np.float32).T
    return out


def kernel(**inputs):
    from concourse import bass_utils
    in_maps, with_bias = _prepare_inputs(inputs)
    nc = _get_program(with_bias)
    r = bass_utils.run_bass_kernel_spmd(nc, in_maps,
                                        core_ids=list(range(NCORES)))
    return np.ascontiguousarray(assemble(r.results))


# revision 16
# speedup vs baseline: 1.2374x; 1.2374x over previous
"""Trainium2 Bass kernel v3: sequence-data-parallel decoder, f32r matmuls.

Same sharding as v2 (8 cores = 2 batch x 4 ranks, core owns token chunks
(r, 7-r) of its batch; full weights streamed per core; one KV AllGather per
layer). v3 keys off two hardware findings:
  - bf16 matmuls cost ~5us each on this stack (ldweights slow path);
    float32r runs at full rate -> every matmul operand is f32r in SBUF.
  - weights stay bf16 in DRAM and are cast-DMA'd (gpsimd SWDGE) to f32
    SBUF, halving HBM traffic vs f32 weights.
  - a DMA that starts waiting on a collective semaphore before it is set
    pays a ~2.5ms poll penalty -> the KV unpack is queued behind the
    layer's weight-stream DMAs on the same (gpsimd) queue.
"""
import os
import sys

sys.path.insert(0, "/opt/trn_rl_repo")

import numpy as np
import ml_dtypes

L, B, T, HID = 4, 2, 1024, 2048
NH, NKV, HD = 16, 4, 128
INTER = 5632
EPS = 1e-6
NCORES, RANKS = 8, 4
KT = HID // 128
NQT = NH * HD // 128
NKVT = NKV * HD // 128
NCT = NQT + 2 * NKVT
NIT = INTER // 128
CH = 128
TOK = 2 * CH
NKC = 8
RG = [[0, 1, 2, 3], [4, 5, 6, 7]]
WCAST = os.environ.get("K3_WCAST", "1") == "1"  # bf16 weights + cast DMA

_CACHE = {}


def _chunks(r):
    return (r, 7 - r)


def _build_program(with_bias, depth_mult=1, coll_mode="ag"):
    import concourse.bacc as bacc
    import concourse.tile as tile
    import concourse.mybir as mybir
    from contextlib import ExitStack

    F32 = mybir.dt.float32
    F32R = mybir.dt.float32r
    BF16 = mybir.dt.bfloat16
    WDT = BF16 if WCAST else F32R
    AF = mybir.ActivationFunctionType
    OP = mybir.AluOpType

    nc = bacc.Bacc("TRN2", target_bir_lowering=False, debug=False,
                   num_devices=NCORES)

    def wdma(out, in_):
        # weight-stream DMA: casting (gpsimd SWDGE) when DRAM is bf16
        if WCAST:
            nc.gpsimd.dma_start(out=out, in_=in_)
        else:
            nc.gpsimd.dma_start(out=out, in_=in_)

    XT = nc.dram_tensor("xt_in", [HID, TOK], F32, kind="ExternalInput")
    WQKV = nc.dram_tensor("wqkv", [L, NCT, 128, KT, 128], WDT, kind="ExternalInput")
    WO = nc.dram_tensor("wo", [L, KT, 128, KT, 128], WDT, kind="ExternalInput")
    WGU = nc.dram_tensor("wgu", [L, NIT, 128, KT, 256], WDT, kind="ExternalInput")
    WD = nc.dram_tensor("wd", [L, KT, 128, NIT, 128], WDT, kind="ExternalInput")
    COST = nc.dram_tensor("cost", [HD, TOK], F32, kind="ExternalInput")
    SINST = nc.dram_tensor("sinst", [HD, TOK], F32, kind="ExternalInput")
    AMASK = nc.dram_tensor("amask", [NKC, 128, TOK], F32, kind="ExternalInput")
    ROT = nc.dram_tensor("rot", [128, 128], F32R, kind="ExternalInput")
    IDT = nc.dram_tensor("idt", [128, 128], F32R, kind="ExternalInput")
    ONES = nc.dram_tensor("ones", [128, 1], F32R, kind="ExternalInput")
    ONESR = nc.dram_tensor("onesr", [1, 128], F32R, kind="ExternalInput")
    NRMW = nc.dram_tensor("nrmw", [128, KT], F32, kind="ExternalInput")
    EPST = nc.dram_tensor("epst", [1, 1], F32, kind="ExternalInput")
    if with_bias:
        QKVB = nc.dram_tensor("qkvb", [L, 128, NCT], F32, kind="ExternalInput")
    OXT = nc.dram_tensor("oxt", [HID, TOK], F32, kind="ExternalOutput")

    with tile.TileContext(nc) as tc, ExitStack() as top:
        persist = top.enter_context(tc.tile_pool(name="persist", bufs=1))
        dram = top.enter_context(tc.tile_pool(name="dram", bufs=2, space="DRAM"))

        xt = persist.tile([128, KT, TOK], F32)
        nc.sync.dma_start(out=xt, in_=XT.ap().rearrange("(k p) t -> p k t", p=128))
        cost = persist.tile([128, TOK], F32)
        nc.sync.dma_start(out=cost, in_=COST.ap())
        sinst = persist.tile([128, TOK], F32)
        nc.sync.dma_start(out=sinst, in_=SINST.ap())
        amask = persist.tile([128, NKC, TOK], F32)
        nc.sync.dma_start(out=amask, in_=AMASK.ap().rearrange("g p t -> p g t"))
        rotm = persist.tile([128, 128], F32R)
        nc.sync.dma_start(out=rotm, in_=ROT.ap())
        idt = persist.tile([128, 128], F32R)
        nc.sync.dma_start(out=idt, in_=IDT.ap())
        ones = persist.tile([128, 1], F32R)
        nc.sync.dma_start(out=ones, in_=ONES.ap())
        onesr = persist.tile([1, 128], F32R)
        nc.sync.dma_start(out=onesr, in_=ONESR.ap())
        nrmw = persist.tile([128, KT], F32)
        nc.sync.dma_start(out=nrmw, in_=NRMW.ap())
        epst = persist.tile([1, 1], F32)
        nc.sync.dma_start(out=epst, in_=EPST.ap())
        if with_bias:
            qkvb = persist.tile([128, L, NCT], F32)
            nc.sync.dma_start(out=qkvb, in_=QKVB.ap().rearrange("l p c -> p l c"))

        def norm_to(pool, psp, xh):
            """xh [128, KT, TOK] f32r = x * rsqrt(mean(x^2)+eps)."""
            var = psp.tile([1, TOK], F32, name="var", bufs=1)
            for k in range(KT):
                sq = pool.tile([128, TOK], F32R, name="sq", bufs=3)
                nc.vector.tensor_tensor(out=sq, in0=xt[:, k, :], in1=xt[:, k, :],
                                        op=OP.mult)
                nc.tensor.matmul(var, ones, sq, start=(k == 0), stop=(k == KT - 1),
                                 skip_group_check=True)
            std = pool.tile([1, TOK], F32, name="std", bufs=1)
            nc.scalar.activation(out=std, in_=var, func=AF.Sqrt,
                                 bias=epst[:, 0:1], scale=1.0 / HID)
            rec = pool.tile([1, TOK], F32R, name="rec", bufs=1)
            with nc.allow_low_precision(reason="f32r bytes are f32"):
                nc.vector.reciprocal(out=rec, in_=std)
            rb = psp.tile([128, TOK], F32, name="rbp", bufs=1)
            nc.tensor.matmul(rb, onesr, rec, start=True, stop=True,
                             skip_group_check=True)
            for k in range(KT):
                nc.vector.tensor_tensor(out=xh[:, k, :], in0=xt[:, k, :],
                                        in1=rb, op=OP.mult)

        for l in [li % L for li in range(L * depth_mult)]:
            with ExitStack() as ls:
                sbL = ls.enter_context(tc.tile_pool(name="sbL", bufs=1))
                ao_scope = ExitStack()
                sbAO = ao_scope.enter_context(tc.tile_pool(name="sbAO", bufs=1))
                aoT = sbAO.tile([128, NQT, TOK], F32R, name="aoT")
                att_scope = ExitStack()
                sbQK = att_scope.enter_context(tc.tile_pool(name="sbQK", bufs=1))
                qf = sbQK.tile([128, NQT, TOK], F32R, name="qf")
                kall = sbQK.tile([128, NKV, RANKS, TOK], F32R, name="kall")
                vall = sbQK.tile([128, NKV, RANKS, TOK], F32R, name="vall")

                # ---------- phase A: norm1 + kv/q proj + rope + AG ----------
                with ExitStack() as ph:
                    sbA = ph.enter_context(tc.tile_pool(name="sbA", bufs=2))
                    psS = ph.enter_context(tc.tile_pool(name="psS", bufs=1, space="PSUM"))
                    psW = ph.enter_context(tc.tile_pool(name="psW", bufs=3, space="PSUM"))
                    psR = ph.enter_context(tc.tile_pool(name="psR", bufs=2, space="PSUM"))

                    xh = sbA.tile([128, KT, TOK], F32R, name="xh", bufs=1)
                    norm_to(sbA, psS, xh)

                    kvpack = sbA.tile([128, NKV, 2, TOK], F32R, name="kvpack", bufs=1)

                    def proj(ct, wtile):
                        ps = psW.tile([128, TOK], F32, name="pqkv", bufs=3)
                        for k in range(KT):
                            nc.tensor.matmul(ps, wtile[:, k, :], xh[:, k, :],
                                             start=(k == 0), stop=(k == KT - 1),
                                             skip_group_check=True)
                        return ps

                    def rope(dst, ps, ct):
                        plain = sbA.tile([128, TOK], F32R, name="plain", bufs=2)
                        if with_bias:
                            nc.vector.tensor_scalar_add(
                                out=plain, in0=ps, scalar1=qkvb[:, l, ct:ct + 1])
                        else:
                            nc.scalar.copy(plain, ps)
                        rps = psR.tile([128, TOK], F32, name="rps", bufs=1)
                        nc.tensor.matmul(rps, rotm, plain, start=True, stop=True,
                                         skip_group_check=True)
                        qc = sbA.tile([128, TOK], F32, name="qc", bufs=2)
                        nc.vector.tensor_tensor(out=qc, in0=plain.bitcast(F32),
                                                in1=cost, op=OP.mult)
                        rs = sbA.tile([128, TOK], F32, name="rs", bufs=2)
                        nc.vector.tensor_tensor(out=rs, in0=rps, in1=sinst, op=OP.mult)
                        nc.vector.tensor_tensor(out=dst, in0=qc, in1=rs, op=OP.add)

                    # k tiles: proj + rope -> kvpack[:, j, 0, :]
                    for j in range(NKVT):
                        ct = NQT + j
                        w = sbA.tile([128, KT, 128], F32R, name="wk", bufs=3)
                        wdma(w, WQKV.ap()[l, ct])
                        ps = proj(ct, w)
                        rope(kvpack[:, j, 0, :], ps, ct)
                    # v tiles: proj + evict + transpose -> kvpack[:, j, 1, :]
                    for j in range(NKVT):
                        ct = NQT + NKVT + j
                        w = sbA.tile([128, KT, 128], F32R, name="wv", bufs=3)
                        wdma(w, WQKV.ap()[l, ct])
                        ps = proj(ct, w)
                        vtmp = sbA.tile([128, TOK], F32R, name="vtmp", bufs=2)
                        if with_bias:
                            nc.vector.tensor_scalar_add(
                                out=vtmp, in0=ps, scalar1=qkvb[:, l, ct:ct + 1])
                        else:
                            nc.scalar.copy(vtmp, ps)
                        for cc in range(2):
                            pv = psR.tile([128, 128], F32R, name="pv", bufs=1)
                            nc.tensor.transpose(
                                pv, vtmp[:, cc * 128:(cc + 1) * 128], idt)
                            nc.scalar.copy(kvpack[:, j, 1, cc * 128:(cc + 1) * 128],
                                           pv.bitcast(F32))

                    # KV exchange: SBUF f32 -> DRAM bf16 (cast) -> AllGather
                    kvout = dram.tile([RANKS, 128, NKV, 2, TOK], BF16, name="kvout",
                                      bufs=2)
                    kvin = dram.tile([128, NKV, 2, TOK], BF16, name="kvin", bufs=2)
                    nc.gpsimd.dma_start(out=kvin, in_=kvpack.bitcast(F32))
                    if coll_mode == "fake":
                        for rr in range(RANKS):
                            nc.gpsimd.dma_start(out=kvout[rr], in_=kvin)
                    elif coll_mode == "none":
                        pass  # timing probe: unpack reads stale kvout
                    else:
                        nc.gpsimd.collective_compute(
                            "AllGather", mybir.AluOpType.bypass, replica_groups=RG,
                            ins=[kvin.opt()], outs=[kvout.opt()])

                    # q tiles + rope; their weight DMAs double as the queue
                    # spacer between AG launch and the unpack below
                    for ct in range(NQT):
                        w = sbA.tile([128, KT, 128], F32R, name="wq", bufs=3)
                        wdma(w, WQKV.ap()[l, ct])
                        ps = proj(ct, w)
                        rope(qf[:, ct, :], ps, ct)

                    # unpack gathered KV (cast bf16 -> f32), first AG waiter
                    for kv in range(NKV):
                        nc.gpsimd.dma_start(
                            out=kall[:, kv], in_=kvout[:, :, kv, 0, :].rearrange(
                                "r p t -> p r t"))
                        nc.gpsimd.dma_start(
                            out=vall[:, kv], in_=kvout[:, :, kv, 1, :].rearrange(
                                "r p t -> p r t"))

                # ---------- phase B: attention ----------
                with ExitStack() as ph:
                    sbB = ph.enter_context(tc.tile_pool(name="sbB", bufs=2))
                    psSc = ph.enter_context(tc.tile_pool(name="psSc", bufs=3, space="PSUM"))
                    psAO = ph.enter_context(tc.tile_pool(name="psAO", bufs=2, space="PSUM"))
                    psSum = ph.enter_context(tc.tile_pool(name="psSum", bufs=2, space="PSUM"))

                    for h in range(NH):
                        kv = h // (NH // NKV)
                        pao = psAO.tile([128, TOK], F32, name="pao", bufs=2)
                        psm = psSum.tile([1, TOK], F32, name="psm", bufs=2)
                        for g in range(NKC):
                            rr, cc = (g, 0) if g < 4 else (7 - g, 1)
                            ktile = kall[:, kv, rr, cc * 128:(cc + 1) * 128]
                            vtile = vall[:, kv, rr, cc * 128:(cc + 1) * 128]
                            sc = psSc.tile([128, TOK], F32, name="sc", bufs=3)
                            nc.tensor.matmul(sc, ktile, qf[:, h, :],
                                             start=True, stop=True,
                                             skip_group_check=True)
                            ex = sbB.tile([128, TOK], F32R, name="ex", bufs=4)
                            nc.scalar.activation(out=ex, in_=sc, func=AF.Exp)
                            nc.vector.tensor_tensor(out=ex, in0=ex.bitcast(F32),
                                                    in1=amask[:, g, :], op=OP.mult)
                            nc.tensor.matmul(pao, vtile, ex,
                                             start=(g == 0), stop=(g == NKC - 1),
                                             skip_group_check=True)
                            nc.tensor.matmul(psm, ones, ex,
                                             start=(g == 0), stop=(g == NKC - 1),
                                             skip_group_check=True)
                        rw = sbB.tile([1, TOK], F32R, name="rw", bufs=2)
                        with nc.allow_low_precision(reason="f32r bytes are f32"):
                            nc.vector.reciprocal(out=rw, in_=psm)
                        rb = psSum.tile([128, TOK], F32, name="rb2", bufs=1)
                        nc.tensor.matmul(rb, onesr, rw, start=True, stop=True,
                                         skip_group_check=True)
                        rbs = sbB.tile([128, TOK], F32, name="rbs", bufs=2)
                        nc.scalar.copy(rbs, rb)
                        nc.vector.tensor_tensor(out=aoT[:, h, :], in0=pao, in1=rbs,
                                                op=OP.mult)
                att_scope.close()

                # ---------- phase C: o-proj + residual ----------
                with ExitStack() as ph:
                    sbC = ph.enter_context(tc.tile_pool(name="sbC", bufs=2))
                    psO = ph.enter_context(tc.tile_pool(name="psO", bufs=2, space="PSUM"))
                    for ho in range(KT):
                        if ho % 4 == 0:
                            woc = sbC.tile([128, 4, KT, 128], F32R, name="woc",
                                           bufs=2)
                            wdma(woc, WO.ap()[l, ho:ho + 4].rearrange(
                                "n p k c -> p n k c"))
                        w = woc[:, ho % 4]
                        po = psO.tile([128, TOK], F32, name="po", bufs=2)
                        for hk in range(NQT):
                            nc.tensor.matmul(po, w[:, hk, :], aoT[:, hk, :],
                                             start=(hk == 0), stop=(hk == NQT - 1),
                                             skip_group_check=True)
                        nc.vector.tensor_tensor(out=xt[:, ho, :], in0=xt[:, ho, :],
                                                in1=po, op=OP.add)
                ao_scope.close()

                # ---------- phase D: norm2 + MLP + residual ----------
                with ExitStack() as ph:
                    sbD = ph.enter_context(tc.tile_pool(name="sbD", bufs=2))
                    psS2 = ph.enter_context(tc.tile_pool(name="psS2", bufs=1, space="PSUM"))
                    psG = ph.enter_context(tc.tile_pool(name="psG", bufs=2, space="PSUM"))
                    psU = ph.enter_context(tc.tile_pool(name="psU", bufs=2, space="PSUM"))
                    psD = ph.enter_context(tc.tile_pool(name="psD", bufs=2, space="PSUM"))

                    xh2 = sbD.tile([128, KT, TOK], F32R, name="xh2", bufs=1)
                    norm_to(sbD, psS2, xh2)
                    mT = sbD.tile([128, NIT, TOK], F32R, name="mT", bufs=1)
                    for ci in range(NIT):
                        if ci % 2 == 0:
                            wguc = sbD.tile([128, 2, KT, 256], F32R, name="wguc",
                                            bufs=2)
                            wdma(wguc, WGU.ap()[l, ci:ci + 2].rearrange(
                                "n p k c -> p n k c"))
                        wgu = wguc[:, ci % 2]
                        pg = psG.tile([128, TOK], F32, name="pg", bufs=2)
                        pu = psU.tile([128, TOK], F32, name="pu", bufs=2)
                        for k in range(KT):
                            nc.tensor.matmul(pg, wgu[:, k, 0:128], xh2[:, k, :],
                                             start=(k == 0), stop=(k == KT - 1),
                                             skip_group_check=True)
                        for k in range(KT):
                            nc.tensor.matmul(pu, wgu[:, k, 128:256], xh2[:, k, :],
                                             start=(k == 0), stop=(k == KT - 1),
                                             skip_group_check=True)
                        gsl = sbD.tile([128, TOK], F32, name="gsl", bufs=2)
                        nc.scalar.activation(out=gsl, in_=pg, func=AF.Silu)
                        nc.vector.tensor_tensor(out=mT[:, ci, :], in0=gsl, in1=pu,
                                                op=OP.mult)
                    for ho in range(KT):
                        wd = sbD.tile([128, NIT, 128], F32R, name="wd_t", bufs=2)
                        wdma(wd, WD.ap()[l, ho])
                        pd = psD.tile([128, TOK], F32, name="pd", bufs=2)
                        for ki in range(NIT):
                            nc.tensor.matmul(pd, wd[:, ki, :], mT[:, ki, :],
                                             start=(ki == 0), stop=(ki == NIT - 1),
                                             skip_group_check=True)
                        nc.vector.tensor_tensor(out=xt[:, ho, :], in0=xt[:, ho, :],
                                                in1=pd, op=OP.add)

        # ---------------- final norm + output ----------------
        with ExitStack() as ph:
            sbF = ph.enter_context(tc.tile_pool(name="sbF", bufs=2))
            psF = ph.enter_context(tc.tile_pool(name="psF", bufs=1, space="PSUM"))
            var = psF.tile([1, TOK], F32, name="var", bufs=1)
            for k in range(KT):
                sq = sbF.tile([128, TOK], F32R, name="sq", bufs=3)
                nc.vector.tensor_tensor(out=sq, in0=xt[:, k, :], in1=xt[:, k, :],
                                        op=OP.mult)
                nc.tensor.matmul(var, ones, sq, start=(k == 0), stop=(k == KT - 1),
                                 skip_group_check=True)
            std = sbF.tile([1, TOK], F32, name="std", bufs=1)
            nc.scalar.activation(out=std, in_=var, func=AF.Sqrt,
                                 bias=epst[:, 0:1], scale=1.0 / HID)
            rec = sbF.tile([1, TOK], F32R, name="rec", bufs=1)
            with nc.allow_low_precision(reason="f32r bytes are f32"):
                nc.vector.reciprocal(out=rec, in_=std)
            rb = psF.tile([128, TOK], F32, name="rbf", bufs=1)
            nc.tensor.matmul(rb, onesr, rec, start=True, stop=True,
                             skip_group_check=True)
            for k in range(KT):
                tmp = sbF.tile([128, TOK], F32, name="tmp", bufs=3)
                nc.vector.tensor_tensor(out=tmp, in0=xt[:, k, :], in1=rb, op=OP.mult)
                ot = sbF.tile([128, TOK], F32, name="ot", bufs=3)
                nc.vector.tensor_scalar_mul(out=ot, in0=tmp, scalar1=nrmw[:, k:k + 1])
                nc.sync.dma_start(out=OXT.ap()[k * 128:(k + 1) * 128, :], in_=ot)

    nc.compile()
    return nc


def _prepare_inputs(inputs):
    g = {k: np.asarray(v) for k, v in inputs.items()}
    qw, kw, vw, ow = g["qw"], g["kw"], g["vw"], g["ow"]
    gatew, upw, downw = g["gatew"], g["upw"], g["downw"]
    ln1w, ln2w, normw = g["ln1w"], g["ln2w"], g["normw"]
    hs, cos, sin = g["hidden_states"], g["cos"], g["sin"]
    qb, kb, vb = g["qb"], g["kb"], g["vb"]

    with_bias = bool(np.any(qb) or np.any(kb) or np.any(vb))
    sc = 1.0 / np.sqrt(HD)
    wdt = ml_dtypes.bfloat16 if WCAST else np.float32

    wqkv = np.empty([L, NCT, 128, KT, 128], wdt)
    wo = np.empty([L, KT, 128, KT, 128], wdt)
    wgu = np.empty([L, NIT, 128, KT, 256], wdt)
    wd = np.empty([L, KT, 128, NIT, 128], wdt)
    qkvb = np.zeros([L, 128, NCT], np.float32)

    def pmajor(wt):
        K = wt.shape[0]
        return np.ascontiguousarray(
            wt.reshape(K // 128, 128, wt.shape[1]).transpose(1, 0, 2))

    for l in range(L):
        qs = (qw[l] * ln1w[l][None, :] * sc).astype(np.float32)
        ks = (kw[l] * ln1w[l][None, :]).astype(np.float32)
        vs = (vw[l] * ln1w[l][None, :]).astype(np.float32)
        gs = (gatew[l] * ln2w[l][None, :]).astype(np.float32)
        us = (upw[l] * ln2w[l][None, :]).astype(np.float32)
        for j in range(NQT):
            wqkv[l, j] = pmajor(qs[j * 128:(j + 1) * 128, :].T).astype(wdt)
            qkvb[l, :, j] = qb[l, j * 128:(j + 1) * 128] * sc
        for j in range(NKVT):
            wqkv[l, NQT + j] = pmajor(ks[j * 128:(j + 1) * 128, :].T).astype(wdt)
            wqkv[l, NQT + NKVT + j] = pmajor(
                vs[j * 128:(j + 1) * 128, :].T).astype(wdt)
            qkvb[l, :, NQT + j] = kb[l, j * 128:(j + 1) * 128]
            qkvb[l, :, NQT + NKVT + j] = vb[l, j * 128:(j + 1) * 128]
        for j in range(KT):
            wo[l, j] = pmajor(ow[l].T[:, j * 128:(j + 1) * 128]).astype(wdt)
        gut = np.empty([HID, 256], np.float32)
        for ci in range(NIT):
            gut[:, 0:128] = gs[ci * 128:(ci + 1) * 128, :].T
            gut[:, 128:256] = us[ci * 128:(ci + 1) * 128, :].T
            wgu[l, ci] = pmajor(gut).astype(wdt)
        for j in range(KT):
            wd[l, j] = pmajor(downw[l].T[:, j * 128:(j + 1) * 128]).astype(wdt)

    rotm = np.zeros([128, 128], np.float32)
    for i in range(64):
        rotm[i, i + 64] = 1.0
        rotm[i + 64, i] = 1.0

    common = {
        "wqkv": wqkv, "wo": wo, "wgu": wgu, "wd": wd,
        "rot": rotm,
        "idt": np.eye(128, dtype=np.float32),
        "ones": np.ones([128, 1], np.float32),
        "onesr": np.ones([1, 128], np.float32),
        "nrmw": np.ascontiguousarray(
            np.asarray(normw, np.float32).reshape(KT, 128).T),
        "epst": np.full([1, 1], EPS, np.float32),
    }
    if with_bias:
        common["qkvb"] = qkvb

    in_maps = []
    for c in range(NCORES):
        b, r = c // RANKS, c % RANKS
        c0, c1 = _chunks(r)
        idx = np.concatenate([np.arange(c0 * CH, (c0 + 1) * CH),
                              np.arange(c1 * CH, (c1 + 1) * CH)])
        m = dict(common)
        m["xt_in"] = np.ascontiguousarray(np.asarray(hs[b], np.float32).T[:, idx])
        m["cost"] = np.ascontiguousarray(np.asarray(cos[b], np.float32).T[:, idx])
        sb_ = np.asarray(sin[b], np.float32)
        m["sinst"] = np.ascontiguousarray(
            np.concatenate([-sb_[idx, :HD // 2].T, sb_[idx, HD // 2:].T], axis=0))
        kpos = np.arange(CH)
        am = np.empty([NKC, 128, TOK], np.float32)
        for gch in range(NKC):
            kabs = gch * CH + kpos
            am[gch] = (kabs[:, None] <= idx[None, :]).astype(np.float32)
        m["amask"] = am
        in_maps.append(m)
    return in_maps, with_bias


def _get_program(with_bias, depth_mult=1, fake_coll=False, coll_mode=None):
    cm = "fake" if fake_coll else (coll_mode or "ag")
    key = ("prog3", with_bias, depth_mult, cm, WCAST)
    if key not in _CACHE:
        _CACHE[key] = _build_program(with_bias, depth_mult, cm)
    return _CACHE[key]


def assemble(results):
    out = np.empty([B, T, HID], np.float32)
    for c in range(NCORES):
        b, r = c // RANKS, c % RANKS
        c0, c1 = _chunks(r)
        idx = np.concatenate([np.arange(c0 * CH, (c0 + 1) * CH),
                              np.arange(c1 * CH, (c1 + 1) * CH)])
        out[b, idx, :] = np.asarray(results[c]["oxt"], np.float32).T
    return out


def kernel(**inputs):
    from concourse import bass_utils
    in_maps, with_bias = _prepare_inputs(inputs)
    nc = _get_program(with_bias)
    r = bass_utils.run_bass_kernel_spmd(nc, in_maps,
                                        core_ids=list(range(NCORES)))
    return np.ascontiguousarray(assemble(r.results))


# revision 18
# speedup vs baseline: 1.2887x; 1.0415x over previous
"""Trainium2 Bass kernel v3: sequence-data-parallel decoder, f32r matmuls.

Same sharding as v2 (8 cores = 2 batch x 4 ranks, core owns token chunks
(r, 7-r) of its batch; full weights streamed per core; one KV AllGather per
layer). v3 keys off two hardware findings:
  - bf16 matmuls cost ~5us each on this stack (ldweights slow path);
    float32r runs at full rate -> every matmul operand is f32r in SBUF.
  - weights stay bf16 in DRAM and are cast-DMA'd (gpsimd SWDGE) to f32
    SBUF, halving HBM traffic vs f32 weights.
  - a DMA that starts waiting on a collective semaphore before it is set
    pays a ~2.5ms poll penalty -> the KV unpack is queued behind the
    layer's weight-stream DMAs on the same (gpsimd) queue.
"""
import os
import sys

sys.path.insert(0, "/opt/trn_rl_repo")

import numpy as np
import ml_dtypes

L, B, T, HID = 4, 2, 1024, 2048
NH, NKV, HD = 16, 4, 128
INTER = 5632
EPS = 1e-6
NCORES, RANKS = 8, 4
KT = HID // 128
NQT = NH * HD // 128
NKVT = NKV * HD // 128
NCT = NQT + 2 * NKVT
NIT = INTER // 128
CH = 128
TOK = 2 * CH
NKC = 8
RG = [[0, 1, 2, 3], [4, 5, 6, 7]]
WCAST = os.environ.get("K3_WCAST", "1") == "1"  # bf16 weights + cast DMA

_CACHE = {}


def _chunks(r):
    return (r, 7 - r)


def _build_program(with_bias, depth_mult=1, coll_mode="ag"):
    import concourse.bacc as bacc
    import concourse.tile as tile
    import concourse.mybir as mybir
    from contextlib import ExitStack

    F32 = mybir.dt.float32
    F32R = mybir.dt.float32r
    BF16 = mybir.dt.bfloat16
    WDT = BF16 if WCAST else F32R
    AF = mybir.ActivationFunctionType
    OP = mybir.AluOpType

    nc = bacc.Bacc("TRN2", target_bir_lowering=False, debug=False,
                   num_devices=NCORES)

    def wdma(out, in_, q=0):
        # weight-stream DMA: casting (gpsimd SWDGE) when DRAM is bf16
        nc.gpsimd.dma_start(out=out, in_=in_)

    XT = nc.dram_tensor("xt_in", [HID, TOK], F32, kind="ExternalInput")
    WQKV = nc.dram_tensor("wqkv", [L, NCT, 128, KT, 128], WDT, kind="ExternalInput")
    WO = nc.dram_tensor("wo", [L, KT, 128, KT, 128], WDT, kind="ExternalInput")
    WGU = nc.dram_tensor("wgu", [L, NIT, 128, KT, 256], WDT, kind="ExternalInput")
    WD = nc.dram_tensor("wd", [L, KT, 128, NIT, 128], WDT, kind="ExternalInput")
    COST = nc.dram_tensor("cost", [HD, TOK], F32, kind="ExternalInput")
    SINST = nc.dram_tensor("sinst", [HD, TOK], F32, kind="ExternalInput")
    AMASK = nc.dram_tensor("amask", [NKC, 128, TOK], F32, kind="ExternalInput")
    ROT = nc.dram_tensor("rot", [128, 128], F32R, kind="ExternalInput")
    IDT = nc.dram_tensor("idt", [128, 128], F32R, kind="ExternalInput")
    ONES = nc.dram_tensor("ones", [128, 1], F32R, kind="ExternalInput")
    ONESR = nc.dram_tensor("onesr", [1, 128], F32R, kind="ExternalInput")
    NRMW = nc.dram_tensor("nrmw", [128, KT], F32, kind="ExternalInput")
    EPST = nc.dram_tensor("epst", [1, 1], F32, kind="ExternalInput")
    if with_bias:
        QKVB = nc.dram_tensor("qkvb", [L, 128, NCT], F32, kind="ExternalInput")
    OXT = nc.dram_tensor("oxt", [HID, TOK], F32, kind="ExternalOutput")

    with tile.TileContext(nc) as tc, ExitStack() as top:
        persist = top.enter_context(tc.tile_pool(name="persist", bufs=1))
        dram = top.enter_context(tc.tile_pool(name="dram", bufs=2, space="DRAM"))

        xt = persist.tile([128, KT, TOK], F32)
        nc.sync.dma_start(out=xt, in_=XT.ap().rearrange("(k p) t -> p k t", p=128))
        cost = persist.tile([128, TOK], F32)
        nc.sync.dma_start(out=cost, in_=COST.ap())
        sinst = persist.tile([128, TOK], F32)
        nc.sync.dma_start(out=sinst, in_=SINST.ap())
        amask = persist.tile([128, NKC, TOK], F32)
        nc.sync.dma_start(out=amask, in_=AMASK.ap().rearrange("g p t -> p g t"))
        rotm = persist.tile([128, 128], F32R)
        nc.sync.dma_start(out=rotm, in_=ROT.ap())
        idt = persist.tile([128, 128], F32R)
        nc.sync.dma_start(out=idt, in_=IDT.ap())
        ones = persist.tile([128, 1], F32R)
        nc.sync.dma_start(out=ones, in_=ONES.ap())
        onesr = persist.tile([1, 128], F32R)
        nc.sync.dma_start(out=onesr, in_=ONESR.ap())
        nrmw = persist.tile([128, KT], F32)
        nc.sync.dma_start(out=nrmw, in_=NRMW.ap())
        epst = persist.tile([1, 1], F32)
        nc.sync.dma_start(out=epst, in_=EPST.ap())
        if with_bias:
            qkvb = persist.tile([128, L, NCT], F32)
            nc.sync.dma_start(out=qkvb, in_=QKVB.ap().rearrange("l p c -> p l c"))

        def norm_to(pool, psp, xh):
            """xh [128, KT, TOK] f32r = x * rsqrt(mean(x^2)+eps)."""
            var = psp.tile([1, TOK], F32, name="var", bufs=1)
            for k in range(KT):
                sq = pool.tile([128, TOK], F32R, name="sq", bufs=3)
                nc.vector.tensor_tensor(out=sq, in0=xt[:, k, :], in1=xt[:, k, :],
                                        op=OP.mult)
                nc.tensor.matmul(var, ones, sq, start=(k == 0), stop=(k == KT - 1),
                                 skip_group_check=True)
            std = pool.tile([1, TOK], F32, name="std", bufs=1)
            nc.scalar.activation(out=std, in_=var, func=AF.Sqrt,
                                 bias=epst[:, 0:1], scale=1.0 / HID)
            rec = pool.tile([1, TOK], F32R, name="rec", bufs=1)
            with nc.allow_low_precision(reason="f32r bytes are f32"):
                nc.vector.reciprocal(out=rec, in_=std)
            rb = psp.tile([128, TOK], F32, name="rbp", bufs=1)
            nc.tensor.matmul(rb, onesr, rec, start=True, stop=True,
                             skip_group_check=True)
            for k in range(KT):
                nc.vector.tensor_tensor(out=xh[:, k, :], in0=xt[:, k, :],
                                        in1=rb, op=OP.mult)

        for l in [li % L for li in range(L * depth_mult)]:
            with ExitStack() as ls:
                sbL = ls.enter_context(tc.tile_pool(name="sbL", bufs=1))
                ao_scope = ExitStack()
                sbAO = ao_scope.enter_context(tc.tile_pool(name="sbAO", bufs=1))
                aoT = sbAO.tile([128, NQT, TOK], F32R, name="aoT")
                att_scope = ExitStack()
                sbQK = att_scope.enter_context(tc.tile_pool(name="sbQK", bufs=1))
                qf = sbQK.tile([128, NQT, TOK], F32R, name="qf")
                kall = sbQK.tile([128, NKV, RANKS, TOK], F32R, name="kall")
                vall = sbQK.tile([128, NKV, RANKS, TOK], F32R, name="vall")

                # ---------- phase A: norm1 + kv/q proj + rope + AG ----------
                with ExitStack() as ph:
                    sbA = ph.enter_context(tc.tile_pool(name="sbA", bufs=2))
                    psS = ph.enter_context(tc.tile_pool(name="psS", bufs=1, space="PSUM"))
                    psW = ph.enter_context(tc.tile_pool(name="psW", bufs=3, space="PSUM"))
                    psR = ph.enter_context(tc.tile_pool(name="psR", bufs=2, space="PSUM"))

                    xh = sbA.tile([128, KT, TOK], F32R, name="xh", bufs=1)
                    norm_to(sbA, psS, xh)

                    kvpack = sbA.tile([128, NKV, 2, TOK], F32R, name="kvpack", bufs=1)

                    def proj(ct, wtile):
                        ps = psW.tile([128, TOK], F32, name="pqkv", bufs=3)
                        for k in range(KT):
                            nc.tensor.matmul(ps, wtile[:, k, :], xh[:, k, :],
                                             start=(k == 0), stop=(k == KT - 1),
                                             skip_group_check=True)
                        return ps

                    def rope(dst, ps, ct):
                        plain = sbA.tile([128, TOK], F32R, name="plain", bufs=2)
                        if with_bias:
                            nc.vector.tensor_scalar_add(
                                out=plain, in0=ps, scalar1=qkvb[:, l, ct:ct + 1])
                        else:
                            nc.scalar.copy(plain, ps)
                        rps = psR.tile([128, TOK], F32, name="rps", bufs=2)
                        nc.tensor.matmul(rps, rotm, plain, start=True, stop=True,
                                         skip_group_check=True)
                        qc = sbA.tile([128, TOK], F32, name="qc", bufs=2)
                        nc.vector.tensor_tensor(out=qc, in0=plain.bitcast(F32),
                                                in1=cost, op=OP.mult)
                        rs = sbA.tile([128, TOK], F32, name="rs", bufs=2)
                        nc.vector.tensor_tensor(out=rs, in0=rps, in1=sinst, op=OP.mult)
                        nc.vector.tensor_tensor(out=dst, in0=qc, in1=rs, op=OP.add)

                    # k tiles: proj + rope -> kvpack[:, j, 0, :]
                    for j in range(NKVT):
                        ct = NQT + j
                        w = sbA.tile([128, KT, 128], F32R, name="wk", bufs=3)
                        wdma(w, WQKV.ap()[l, ct])
                        ps = proj(ct, w)
                        rope(kvpack[:, j, 0, :], ps, ct)
                    # v tiles: proj + evict + transpose -> kvpack[:, j, 1, :]
                    for j in range(NKVT):
                        ct = NQT + NKVT + j
                        w = sbA.tile([128, KT, 128], F32R, name="wv", bufs=3)
                        wdma(w, WQKV.ap()[l, ct])
                        ps = proj(ct, w)
                        vtmp = sbA.tile([128, TOK], F32R, name="vtmp", bufs=2)
                        if with_bias:
                            nc.vector.tensor_scalar_add(
                                out=vtmp, in0=ps, scalar1=qkvb[:, l, ct:ct + 1])
                        else:
                            nc.scalar.copy(vtmp, ps)
                        for cc in range(2):
                            pv = psR.tile([128, 128], F32R, name="pv", bufs=1)
                            nc.tensor.transpose(
                                pv, vtmp[:, cc * 128:(cc + 1) * 128], idt)
                            nc.scalar.copy(kvpack[:, j, 1, cc * 128:(cc + 1) * 128],
                                           pv.bitcast(F32))

                    # KV exchange: SBUF f32 -> DRAM bf16 (cast) -> AllGather
                    kvout = dram.tile([RANKS, 128, NKV, 2, TOK], BF16, name="kvout",
                                      bufs=2)
                    kvin = dram.tile([128, NKV, 2, TOK], BF16, name="kvin", bufs=2)
                    nc.gpsimd.dma_start(out=kvin, in_=kvpack.bitcast(F32))
                    if coll_mode == "fake":
                        for rr in range(RANKS):
                            nc.gpsimd.dma_start(out=kvout[rr], in_=kvin)
                    elif coll_mode == "none":
                        pass  # timing probe: unpack reads stale kvout
                    else:
                        nc.gpsimd.collective_compute(
                            "AllGather", mybir.AluOpType.bypass, replica_groups=RG,
                            ins=[kvin.opt()], outs=[kvout.opt()])

                    # q tiles + rope; their weight DMAs double as the queue
                    # spacer between AG launch and the unpack below
                    for ct in range(NQT):
                        w = sbA.tile([128, KT, 128], F32R, name="wq", bufs=3)
                        wdma(w, WQKV.ap()[l, ct])
                        ps = proj(ct, w)
                        rope(qf[:, ct, :], ps, ct)

                    # unpack gathered KV (cast bf16 -> f32), first AG waiter
                    for kv in range(NKV):
                        nc.gpsimd.dma_start(
                            out=kall[:, kv], in_=kvout[:, :, kv, 0, :].rearrange(
                                "r p t -> p r t"))
                        nc.gpsimd.dma_start(
                            out=vall[:, kv], in_=kvout[:, :, kv, 1, :].rearrange(
                                "r p t -> p r t"))

                # ---------- phase B: attention ----------
                with ExitStack() as ph:
                    sbB = ph.enter_context(tc.tile_pool(name="sbB", bufs=2))
                    psSc = ph.enter_context(tc.tile_pool(name="psSc", bufs=3, space="PSUM"))
                    psAO = ph.enter_context(tc.tile_pool(name="psAO", bufs=2, space="PSUM"))
                    psSum = ph.enter_context(tc.tile_pool(name="psSum", bufs=2, space="PSUM"))

                    for h in range(NH):
                        kv = h // (NH // NKV)
                        pao = psAO.tile([128, TOK], F32, name="pao", bufs=2)
                        psm = psSum.tile([1, TOK], F32, name="psm", bufs=2)
                        for g in range(NKC):
                            rr, cc = (g, 0) if g < 4 else (7 - g, 1)
                            ktile = kall[:, kv, rr, cc * 128:(cc + 1) * 128]
                            vtile = vall[:, kv, rr, cc * 128:(cc + 1) * 128]
                            sc = psSc.tile([128, TOK], F32, name="sc", bufs=3)
                            nc.tensor.matmul(sc, ktile, qf[:, h, :],
                                             start=True, stop=True,
                                             skip_group_check=True)
                            ex = sbB.tile([128, TOK], F32R, name="ex", bufs=4)
                            nc.scalar.activation(out=ex, in_=sc, func=AF.Exp)
                            nc.vector.tensor_tensor(out=ex, in0=ex.bitcast(F32),
                                                    in1=amask[:, g, :], op=OP.mult)
                            nc.tensor.matmul(pao, vtile, ex,
                                             start=(g == 0), stop=(g == NKC - 1),
                                             skip_group_check=True)
                            nc.tensor.matmul(psm, ones, ex,
                                             start=(g == 0), stop=(g == NKC - 1),
                                             skip_group_check=True)
                        rw = sbB.tile([1, TOK], F32R, name="rw", bufs=2)
                        with nc.allow_low_precision(reason="f32r bytes are f32"):
                            nc.vector.reciprocal(out=rw, in_=psm)
                        rb = psSum.tile([128, TOK], F32, name="rb2", bufs=1)
                        nc.tensor.matmul(rb, onesr, rw, start=True, stop=True,
                                         skip_group_check=True)
                        rbs = sbB.tile([128, TOK], F32, name="rbs", bufs=2)
                        nc.scalar.copy(rbs, rb)
                        nc.vector.tensor_tensor(out=aoT[:, h, :], in0=pao, in1=rbs,
                                                op=OP.mult)
                att_scope.close()

                # ---------- phase C: o-proj + residual ----------
                with ExitStack() as ph:
                    sbC = ph.enter_context(tc.tile_pool(name="sbC", bufs=2))
                    psO = ph.enter_context(tc.tile_pool(name="psO", bufs=2, space="PSUM"))
                    for ho in range(KT):
                        if ho % 4 == 0:
                            woc = sbC.tile([128, 4, KT, 128], F32R, name="woc",
                                           bufs=2)
                            wdma(woc, WO.ap()[l, ho:ho + 4].rearrange(
                                "n p k c -> p n k c"), q=1)
                        w = woc[:, ho % 4]
                        po = psO.tile([128, TOK], F32, name="po", bufs=2)
                        for hk in range(NQT):
                            nc.tensor.matmul(po, w[:, hk, :], aoT[:, hk, :],
                                             start=(hk == 0), stop=(hk == NQT - 1),
                                             skip_group_check=True)
                        nc.vector.tensor_tensor(out=xt[:, ho, :], in0=xt[:, ho, :],
                                                in1=po, op=OP.add)
                ao_scope.close()

                # ---------- phase D: norm2 + MLP + residual ----------
                with ExitStack() as ph:
                    sbD = ph.enter_context(tc.tile_pool(name="sbD", bufs=2))
                    psS2 = ph.enter_context(tc.tile_pool(name="psS2", bufs=1, space="PSUM"))
                    psG = ph.enter_context(tc.tile_pool(name="psG", bufs=2, space="PSUM"))
                    psU = ph.enter_context(tc.tile_pool(name="psU", bufs=2, space="PSUM"))
                    psD = ph.enter_context(tc.tile_pool(name="psD", bufs=2, space="PSUM"))

                    xh2 = sbD.tile([128, KT, TOK], F32R, name="xh2", bufs=1)
                    norm_to(sbD, psS2, xh2)
                    mT = sbD.tile([128, NIT, TOK], F32R, name="mT", bufs=1)
                    for ci in range(NIT):
                        if ci % 2 == 0:
                            wguc = sbD.tile([128, 2, KT, 256], F32R, name="wguc",
                                            bufs=2)
                            wdma(wguc, WGU.ap()[l, ci:ci + 2].rearrange(
                                "n p k c -> p n k c"), q=1)
                        wgu = wguc[:, ci % 2]
                        pg = psG.tile([128, TOK], F32, name="pg", bufs=2)
                        pu = psU.tile([128, TOK], F32, name="pu", bufs=2)
                        for k in range(KT):
                            nc.tensor.matmul(pg, wgu[:, k, 0:128], xh2[:, k, :],
                                             start=(k == 0), stop=(k == KT - 1),
                                             skip_group_check=True)
                        for k in range(KT):
                            nc.tensor.matmul(pu, wgu[:, k, 128:256], xh2[:, k, :],
                                             start=(k == 0), stop=(k == KT - 1),
                                             skip_group_check=True)
                        gsl = sbD.tile([128, TOK], F32, name="gsl", bufs=2)
                        nc.scalar.activation(out=gsl, in_=pg, func=AF.Silu)
                        nc.vector.tensor_tensor(out=mT[:, ci, :], in0=gsl, in1=pu,
                                                op=OP.mult)
                    for ho in range(KT):
                        wd = sbD.tile([128, NIT, 128], F32R, name="wd_t", bufs=2)
                        wdma(wd, WD.ap()[l, ho], q=1)
                        pd = psD.tile([128, TOK], F32, name="pd", bufs=2)
                        for ki in range(NIT):
                            nc.tensor.matmul(pd, wd[:, ki, :], mT[:, ki, :],
                                             start=(ki == 0), stop=(ki == NIT - 1),
                                             skip_group_check=True)
                        nc.vector.tensor_tensor(out=xt[:, ho, :], in0=xt[:, ho, :],
                                                in1=pd, op=OP.add)

        # ---------------- final norm + output ----------------
        with ExitStack() as ph:
            sbF = ph.enter_context(tc.tile_pool(name="sbF", bufs=2))
            psF = ph.enter_context(tc.tile_pool(name="psF", bufs=1, space="PSUM"))
            var = psF.tile([1, TOK], F32, name="var", bufs=1)
            for k in range(KT):
                sq = sbF.tile([128, TOK], F32R, name="sq", bufs=3)
                nc.vector.tensor_tensor(out=sq, in0=xt[:, k, :], in1=xt[:, k, :],
                                        op=OP.mult)
                nc.tensor.matmul(var, ones, sq, start=(k == 0), stop=(k == KT - 1),
                                 skip_group_check=True)
            std = sbF.tile([1, TOK], F32, name="std", bufs=1)
            nc.scalar.activation(out=std, in_=var, func=AF.Sqrt,
                                 bias=epst[:, 0:1], scale=1.0 / HID)
            rec = sbF.tile([1, TOK], F32R, name="rec", bufs=1)
            with nc.allow_low_precision(reason="f32r bytes are f32"):
                nc.vector.reciprocal(out=rec, in_=std)
            rb = psF.tile([128, TOK], F32, name="rbf", bufs=1)
            nc.tensor.matmul(rb, onesr, rec, start=True, stop=True,
                             skip_group_check=True)
            for k in range(KT):
                tmp = sbF.tile([128, TOK], F32, name="tmp", bufs=3)
                nc.vector.tensor_tensor(out=tmp, in0=xt[:, k, :], in1=rb, op=OP.mult)
                ot = sbF.tile([128, TOK], F32, name="ot", bufs=3)
                nc.vector.tensor_scalar_mul(out=ot, in0=tmp, scalar1=nrmw[:, k:k + 1])
                nc.sync.dma_start(out=OXT.ap()[k * 128:(k + 1) * 128, :], in_=ot)

    nc.compile()
    return nc


def _prepare_inputs(inputs):
    g = {k: np.asarray(v) for k, v in inputs.items()}
    qw, kw, vw, ow = g["qw"], g["kw"], g["vw"], g["ow"]
    gatew, upw, downw = g["gatew"], g["upw"], g["downw"]
    ln1w, ln2w, normw = g["ln1w"], g["ln2w"], g["normw"]
    hs, cos, sin = g["hidden_states"], g["cos"], g["sin"]
    qb, kb, vb = g["qb"], g["kb"], g["vb"]

    with_bias = bool(np.any(qb) or np.any(kb) or np.any(vb))
    sc = 1.0 / np.sqrt(HD)
    wdt = ml_dtypes.bfloat16 if WCAST else np.float32

    wqkv = np.empty([L, NCT, 128, KT, 128], wdt)
    wo = np.empty([L, KT, 128, KT, 128], wdt)
    wgu = np.empty([L, NIT, 128, KT, 256], wdt)
    wd = np.empty([L, KT, 128, NIT, 128], wdt)
    qkvb = np.zeros([L, 128, NCT], np.float32)

    def pmajor(wt):
        K = wt.shape[0]
        return np.ascontiguousarray(
            wt.reshape(K // 128, 128, wt.shape[1]).transpose(1, 0, 2))

    for l in range(L):
        qs = (qw[l] * ln1w[l][None, :] * sc).astype(np.float32)
        ks = (kw[l] * ln1w[l][None, :]).astype(np.float32)
        vs = (vw[l] * ln1w[l][None, :]).astype(np.float32)
        gs = (gatew[l] * ln2w[l][None, :]).astype(np.float32)
        us = (upw[l] * ln2w[l][None, :]).astype(np.float32)
        for j in range(NQT):
            wqkv[l, j] = pmajor(qs[j * 128:(j + 1) * 128, :].T).astype(wdt)
            qkvb[l, :, j] = qb[l, j * 128:(j + 1) * 128] * sc
        for j in range(NKVT):
            wqkv[l, NQT + j] = pmajor(ks[j * 128:(j + 1) * 128, :].T).astype(wdt)
            wqkv[l, NQT + NKVT + j] = pmajor(
                vs[j * 128:(j + 1) * 128, :].T).astype(wdt)
            qkvb[l, :, NQT + j] = kb[l, j * 128:(j + 1) * 128]
            qkvb[l, :, NQT + NKVT + j] = vb[l, j * 128:(j + 1) * 128]
        for j in range(KT):
            wo[l, j] = pmajor(ow[l].T[:, j * 128:(j + 1) * 128]).astype(wdt)
        gut = np.empty([HID, 256], np.float32)
        for ci in range(NIT):
            gut[:, 0:128] = gs[ci * 128:(ci + 1) * 128, :].T
            gut[:, 128:256] = us[ci * 128:(ci + 1) * 128, :].T
            wgu[l, ci] = pmajor(gut).astype(wdt)
        for j in range(KT):
            wd[l, j] = pmajor(downw[l].T[:, j * 128:(j + 1) * 128]).astype(wdt)

    rotm = np.zeros([128, 128], np.float32)
    for i in range(64):
        rotm[i, i + 64] = 1.0
        rotm[i + 64, i] = 1.0

    common = {
        "wqkv": wqkv, "wo": wo, "wgu": wgu, "wd": wd,
        "rot": rotm,
        "idt": np.eye(128, dtype=np.float32),
        "ones": np.ones([128, 1], np.float32),
        "onesr": np.ones([1, 128], np.float32),
        "nrmw": np.ascontiguousarray(
            np.asarray(normw, np.float32).reshape(KT, 128).T),
        "epst": np.full([1, 1], EPS, np.float32),
    }
    if with_bias:
        common["qkvb"] = qkvb

    in_maps = []
    for c in range(NCORES):
        b, r = c // RANKS, c % RANKS
        c0, c1 = _chunks(r)
        idx = np.concatenate([np.arange(c0 * CH, (c0 + 1) * CH),
                              np.arange(c1 * CH, (c1 + 1) * CH)])
        m = dict(common)
        m["xt_in"] = np.ascontiguousarray(np.asarray(hs[b], np.float32).T[:, idx])
        m["cost"] = np.ascontiguousarray(np.asarray(cos[b], np.float32).T[:, idx])
        sb_ = np.asarray(sin[b], np.float32)
        m["sinst"] = np.ascontiguousarray(
            np.concatenate([-sb_[idx, :HD // 2].T, sb_[idx, HD // 2:].T], axis=0))
        kpos = np.arange(CH)
        am = np.empty([NKC, 128, TOK], np.float32)
        for gch in range(NKC):
            kabs = gch * CH + kpos
            am[gch] = (kabs[:, None] <= idx[None, :]).astype(np.float32)
        m["amask"] = am
        in_maps.append(m)
    return in_maps, with_bias


def _get_program(with_bias, depth_mult=1, fake_coll=False, coll_mode=None):
    cm = "fake" if fake_coll else (coll_mode or "ag")
    key = ("prog3", with_bias, depth_mult, cm, WCAST)
    if key not in _CACHE:
        _CACHE[key] = _build_program(with_bias, depth_mult, cm)
    return _CACHE[key]


def assemble(results):
    out = np.empty([B, T, HID], np.float32)
    for c in range(NCORES):
        b, r = c // RANKS, c % RANKS
        c0, c1 = _chunks(r)
        idx = np.concatenate([np.arange(c0 * CH, (c0 + 1) * CH),
                              np.arange(c1 * CH, (c1 + 1) * CH)])
        out[b, idx, :] = np.asarray(results[c]["oxt"], np.float32).T
    return out


def kernel(**inputs):
    from concourse import bass_utils
    in_maps, with_bias = _prepare_inputs(inputs)
    nc = _get_program(with_bias)
    r = bass_utils.run_bass_kernel_spmd(nc, in_maps,
                                        core_ids=list(range(NCORES)))
    return np.ascontiguousarray(assemble(r.results))
